# revision 1
# baseline (speedup 1.0000x reference)
"""Trainium2 Bass kernel for nn_ActorMultiHead (moe_routing).

Strategy
--------
The reference runs every role head on every token (dense form of a masked
dispatch) and then selects the row matching the token's role; tokens whose
role >= NUM_ROLES contribute exactly 0.  We implement the sparse dispatch:

  * Host: flatten [B, A] tokens, drop role>=2 tokens (their output is 0),
    sort the rest by role, pack them into 512-token single-role tiles, and
    assign whole tiles to cores so every core serves exactly one role.
    The one-hot input contribution then folds into the layer-0 bias and the
    per-role heads become per-core weight *data* — the SPMD graph is
    role-agnostic.
  * Device (per core, 6 tiles x 512 tokens): feature-major activations
    [feat_part, token_free] so no transposes are needed anywhere:
        h0 = relu(W0a^T x + b0');  h1 = relu(W1^T h0 + b1)
        h2 = relu(W2^T h1 + b2);   z  = relu(hW1^T h2 + hb1)
        mean = tanh(hW2^T z + hb2)
        lp   = sum_k -0.5*((mean-a)*inv_std)^2 + c
             = matmul(lhsT=-0.5*inv_std^2, (mean-a)^2) + c
    All matmuls run as float32r (full fp32 storage, ~bf16-rate PE).
  * Host: scatter per-tile outputs back to original token positions.

For timing, a variant graph wraps the whole per-core compute in a
constant-bound For_i loop (set `kernel.nrep = R`), so a test harness can
measure HW time as (wall(nrep=R) - wall(nrep=1)) / (R-1).
"""

import math

import numpy as np

# -- problem constants (from the problem statement, hardcoded) ---------------
B, A = 2048, 16
OBS_DIM, HIDDEN, ACTION_DIM = 128, 1024, 8
NUM_ROLES = 2
AGENT_ID_DIM = NUM_ROLES
H2 = HIDDEN // 2
LOG_2PI = math.log(2.0 * math.pi)

N_CORES = 8
NT = 512          # tokens per tile (matmul moving free dim)
CT = 6            # tiles per core (fixed compile-time shape)
C = NT * CT       # tokens per core per batch
KH = HIDDEN // 128    # 8 k/m tiles of hidden
KZ = H2 // 128        # 4 k/m tiles of the head hidden

_GRAPHS = {}  # repeats -> compiled graph, built once per process


def _build_graph(repeats=1):
    import concourse.bass as bass
    import concourse.tile as tile
    from concourse import bacc, mybir

    f32 = mybir.dt.float32
    f32r = mybir.dt.float32r
    Act = mybir.ActivationFunctionType

    nc = bacc.Bacc(None, target_bir_lowering=False)

    xT = nc.declare_dram_parameter("xT", [128, C], f32r, isOutput=False)
    aT = nc.declare_dram_parameter("aT", [ACTION_DIM, C], f32, isOutput=False)
    w0 = nc.declare_dram_parameter("w0", [128, HIDDEN], f32r, isOutput=False)
    b0 = nc.declare_dram_parameter("b0", [128, KH], f32, isOutput=False)
    w1 = nc.declare_dram_parameter("w1", [128, KH, HIDDEN], f32r, isOutput=False)
    b1 = nc.declare_dram_parameter("b1", [128, KH], f32, isOutput=False)
    w2 = nc.declare_dram_parameter("w2", [128, KH, HIDDEN], f32r, isOutput=False)
    b2 = nc.declare_dram_parameter("b2", [128, KH], f32, isOutput=False)
    hw1 = nc.declare_dram_parameter("hw1", [128, KH, H2], f32r, isOutput=False)
    hb1 = nc.declare_dram_parameter("hb1", [128, KZ], f32, isOutput=False)
    hw2 = nc.declare_dram_parameter("hw2", [128, KZ, ACTION_DIM], f32r, isOutput=False)
    hb2 = nc.declare_dram_parameter("hb2", [ACTION_DIM, 1], f32, isOutput=False)
    wred = nc.declare_dram_parameter("wred", [ACTION_DIM, 1], f32r, isOutput=False)
    cc = nc.declare_dram_parameter("cc", [1, 1], f32, isOutput=False)
    out = nc.declare_dram_parameter("out", [CT, NT], f32, isOutput=True)

    with tile.TileContext(nc) as tc:
        with (
            tc.tile_pool(name="consts", bufs=1) as consts,
            tc.tile_pool(name="acts", bufs=1) as acts,
            tc.tile_pool(name="h0p", bufs=2) as h0p,
            tc.tile_pool(name="small", bufs=3) as small,
            tc.tile_pool(name="psum", bufs=4, space="PSUM") as psum,
            tc.tile_pool(name="psmall", bufs=2, space="PSUM") as psmall,
        ):
            # resident weights / biases
            w0_sb = consts.tile([128, HIDDEN], f32r)
            nc.sync.dma_start(w0_sb[:], w0[:])
            w1_sb = consts.tile([128, KH, HIDDEN], f32r)
            nc.sync.dma_start(w1_sb[:], w1[:])
            w2_sb = consts.tile([128, KH, HIDDEN], f32r)
            nc.sync.dma_start(w2_sb[:], w2[:])
            hw1_sb = consts.tile([128, KH, H2], f32r)
            nc.sync.dma_start(hw1_sb[:], hw1[:])
            hw2_sb = consts.tile([128, KZ, ACTION_DIM], f32r)
            nc.sync.dma_start(hw2_sb[:], hw2[:])
            b0_sb = consts.tile([128, KH], f32)
            nc.sync.dma_start(b0_sb[:], b0[:])
            b1_sb = consts.tile([128, KH], f32)
            nc.sync.dma_start(b1_sb[:], b1[:])
            b2_sb = consts.tile([128, KH], f32)
            nc.sync.dma_start(b2_sb[:], b2[:])
            hb1_sb = consts.tile([128, KZ], f32)
            nc.sync.dma_start(hb1_sb[:], hb1[:])
            hb2_sb = consts.tile([ACTION_DIM, 1], f32)
            nc.sync.dma_start(hb2_sb[:], hb2[:])
            wred_sb = consts.tile([ACTION_DIM, 1], f32r)
            nc.sync.dma_start(wred_sb[:], wred[:])
            cc_sb = consts.tile([1, 1], f32)
            nc.sync.dma_start(cc_sb[:], cc[:])

            xT_sb = consts.tile([128, C], f32r)
            nc.sync.dma_start(xT_sb[:], xT[:])
            aT_sb = consts.tile([ACTION_DIM, C], f32)
            nc.sync.dma_start(aT_sb[:], aT[:])

            from contextlib import nullcontext

            loop_cm = tc.For_i(0, repeats, 1) if repeats > 1 else nullcontext()
            with loop_cm:
                # tail of tile t-1 is emitted after the matmul body of tile t
                # so the PE never waits on the DVE->lp-matmul chain
                pending = []

                def emit_tail(t, psm):
                    ts = bass.ts(t, NT)
                    mean = small.tile([ACTION_DIM, NT], f32, tag="mean")
                    nc.scalar.activation(mean[:], psm[:], Act.Tanh, bias=hb2_sb[:, 0:1])
                    d = small.tile([ACTION_DIM, NT], f32, tag="d")
                    nc.vector.tensor_sub(d[:], mean[:], aT_sb[:, ts])
                    sq = small.tile([ACTION_DIM, NT], f32r, tag="sq")
                    nc.vector.tensor_mul(sq[:], d[:], d[:])
                    pl = psmall.tile([1, NT], f32, tag="pl")
                    nc.tensor.matmul(
                        pl[:], wred_sb[:], sq[:], start=True, stop=True
                    )
                    o = small.tile([1, NT], f32, tag="o")
                    nc.vector.tensor_scalar_add(o[:], pl[:], cc_sb[0:1, 0:1])
                    nc.sync.dma_start(out[t : t + 1, :], o[:])

                for t in range(CT):
                    ts = bass.ts(t, NT)

                    # layer 0: h0[m] = relu(w0[:, m]^T @ x + b0')
                    h0 = h0p.tile([128, KH, NT], f32r, tag="h0")
                    for m in range(KH):
                        ps = psum.tile([128, NT], f32, tag="ps")
                        nc.tensor.matmul(
                            ps[:], w0_sb[:, bass.ts(m, 128)], xT_sb[:, ts],
                            start=True, stop=True,
                        )
                        nc.scalar.activation(
                            h0[:, m, :], ps[:], Act.Relu, bias=b0_sb[:, m : m + 1]
                        )

                    # layers 1, 2
                    h1 = acts.tile([128, KH, NT], f32r, tag="h1")
                    for m in range(KH):
                        ps = psum.tile([128, NT], f32, tag="ps")
                        for k in range(KH):
                            nc.tensor.matmul(
                                ps[:],
                                w1_sb[:, k, bass.ts(m, 128)],
                                h0[:, k, :],
                                start=(k == 0), stop=(k == KH - 1),
                            )
                        nc.scalar.activation(
                            h1[:, m, :], ps[:], Act.Relu, bias=b1_sb[:, m : m + 1]
                        )

                    h2 = acts.tile([128, KH, NT], f32r, tag="h2")
                    for m in range(KH):
                        ps = psum.tile([128, NT], f32, tag="ps")
                        for k in range(KH):
                            nc.tensor.matmul(
                                ps[:],
                                w2_sb[:, k, bass.ts(m, 128)],
                                h1[:, k, :],
                                start=(k == 0), stop=(k == KH - 1),
                            )
                        nc.scalar.activation(
                            h2[:, m, :], ps[:], Act.Relu, bias=b2_sb[:, m : m + 1]
                        )

                    # head layer 1: z = relu(hw1^T h2 + hb1)   [512 feats]
                    z = acts.tile([128, KZ, NT], f32r, tag="z")
                    for m in range(KZ):
                        ps = psum.tile([128, NT], f32, tag="ps")
                        for k in range(KH):
                            nc.tensor.matmul(
                                ps[:],
                                hw1_sb[:, k, bass.ts(m, 128)],
                                h2[:, k, :],
                                start=(k == 0), stop=(k == KH - 1),
                            )
                        nc.scalar.activation(
                            z[:, m, :], ps[:], Act.Relu, bias=hb1_sb[:, m : m + 1]
                        )

                    # head layer 2: mean = tanh(hw2^T z + hb2)   [8, NT]
                    psm = psmall.tile([ACTION_DIM, NT], f32, tag="pm")
                    for k in range(KZ):
                        nc.tensor.matmul(
                            psm[:], hw2_sb[:, k, :], z[:, k, :],
                            start=(k == 0), stop=(k == KZ - 1),
                        )
                    pending.append((t, psm))
                    if t > 0:
                        emit_tail(*pending.pop(0))
                if pending:
                    emit_tail(*pending.pop(0))

    nc.compile()
    return nc


def _get_graph(repeats=1):
    if repeats not in _GRAPHS:
        _GRAPHS[repeats] = _build_graph(repeats)
    return _GRAPHS[repeats]


def _round_f32r(a):
    """Round fp32 to the PE's fp32r format (11-bit mantissa, low 12 bits 0)."""
    b = np.ascontiguousarray(a, dtype=np.float32).view(np.uint32)
    lsb = (b >> np.uint32(12)) & np.uint32(1)
    out = (b + np.uint32(0x7FF) + lsb) & np.uint32(0xFFFFF000)
    return out.view(np.float32)


def _pack(obs_f, act_f, tok_pad):
    """Gather+transpose per-core token data: [128, C] and [8, C] f32."""
    xT = np.ascontiguousarray(obs_f[tok_pad].T)
    aT = np.ascontiguousarray(act_f[tok_pad].T)
    return xT, aT


def _role_consts(r, W0, b0, log_stds):
    """Per-role derived weights: fused layer-0 bias, reduce weights, constant."""
    b0p = b0 + W0[OBS_DIM + r]
    inv_std = np.exp(-log_stds[r]).astype(np.float64)
    wred = (-0.5 * inv_std * inv_std).astype(np.float32)
    c = np.float32(-np.sum(log_stds[r]) - 0.5 * LOG_2PI * ACTION_DIM)
    return b0p.astype(np.float32), wred, c


def kernel(
    obs, role_ids, actions,
    W0, b0, W1, b1, W2, b2,
    hW1, hb1, hW2, hb2, log_stds,
):
    from concourse.bass_utils import run_bass_kernel_spmd

    obs = np.asarray(obs, dtype=np.float32)
    role_ids = np.asarray(role_ids)
    actions = np.asarray(actions, dtype=np.float32)
    W0 = np.asarray(W0, dtype=np.float32)
    b0 = np.asarray(b0, dtype=np.float32)
    W1 = np.asarray(W1, dtype=np.float32)
    b1 = np.asarray(b1, dtype=np.float32)
    W2 = np.asarray(W2, dtype=np.float32)
    b2 = np.asarray(b2, dtype=np.float32)
    hW1 = np.asarray(hW1, dtype=np.float32)
    hb1 = np.asarray(hb1, dtype=np.float32)
    hW2 = np.asarray(hW2, dtype=np.float32)
    hb2 = np.asarray(hb2, dtype=np.float32)
    log_stds = np.asarray(log_stds, dtype=np.float32)

    nb, na = role_ids.shape
    obs_f = _round_f32r(obs.reshape(-1, OBS_DIM))
    act_f = actions.reshape(-1, ACTION_DIM)
    roles_f = role_ids.reshape(-1)
    n_tok = roles_f.shape[0]

    # ---- tile lists per role (token index + scatter destination) ----------
    tiles = []  # (role, tok_idx[NT], dst[NT] with -1 for padding)
    for r in range(NUM_ROLES):
        idx = np.nonzero(roles_f == r)[0]
        n = idx.shape[0]
        for s in range(0, n, NT):
            chunk = idx[s : s + NT]
            tok = np.zeros(NT, dtype=np.int64)
            dst = np.full(NT, -1, dtype=np.int64)
            tok[: chunk.shape[0]] = chunk
            dst[: chunk.shape[0]] = chunk
            tiles.append((r, tok, dst))

    out_full = np.zeros(n_tok, dtype=np.float32)
    if not tiles:
        return out_full.reshape(nb, na)

    # ---- static per-role weight payloads ----------------------------------
    w0_dev = _round_f32r(W0[:OBS_DIM])                                # [128,1024]
    w1_dev = _round_f32r(W1.reshape(KH, 128, HIDDEN).transpose(1, 0, 2))
    w2_dev = _round_f32r(W2.reshape(KH, 128, HIDDEN).transpose(1, 0, 2))
    b1_dev = np.ascontiguousarray(b1.reshape(KH, 128).T)
    b2_dev = np.ascontiguousarray(b2.reshape(KH, 128).T)
    role_payload = {}
    for r in range(NUM_ROLES):
        b0p, wred, c = _role_consts(r, W0, b0, log_stds)
        role_payload[r] = dict(
            w0=w0_dev,
            b0=np.ascontiguousarray(b0p.reshape(KH, 128).T),
            w1=w1_dev, b1=b1_dev, w2=w2_dev, b2=b2_dev,
            hw1=_round_f32r(hW1[r].reshape(KH, 128, H2).transpose(1, 0, 2)),
            hb1=np.ascontiguousarray(hb1[r].reshape(KZ, 128).T),
            hw2=_round_f32r(hW2[r].reshape(KZ, 128, ACTION_DIM).transpose(1, 0, 2)),
            hb2=np.ascontiguousarray(hb2[r].reshape(ACTION_DIM, 1)),
            wred=_round_f32r(wred.reshape(ACTION_DIM, 1)),
            cc=np.full((1, 1), c, dtype=np.float32),
        )

    nc = _get_graph(int(getattr(kernel, "nrep", 1)))

    # ---- pack tiles into batches of N_CORES cores x CT single-role tiles --
    # Cores within a batch each take CT tiles of one role.
    batches = []  # list of per-core lists of (tok, dst) + role
    i = 0
    while i < len(tiles):
        cores = []
        for _ in range(N_CORES):
            if i >= len(tiles):
                # replicate an empty dummy core (role 0, token 0, no scatter)
                cores.append((0, []))
                continue
            role = tiles[i][0]
            group = []
            while i < len(tiles) and tiles[i][0] == role and len(group) < CT:
                group.append(tiles[i][1:])
                i += 1
            cores.append((role, group))
        batches.append(cores)

    for cores in batches:
        in_maps = []
        scatter = []  # (core, flat dst array[C])
        for ci, (role, group) in enumerate(cores):
            toks = [g[0] for g in group]
            dsts = [g[1] for g in group]
            while len(toks) < CT:
                toks.append(np.zeros(NT, dtype=np.int64))
                dsts.append(np.full(NT, -1, dtype=np.int64))
            tok_pad = np.concatenate(toks)
            dst_pad = np.concatenate(dsts)
            xT_c, aT_c = _pack(obs_f, act_f, tok_pad)
            m = dict(role_payload[role])
            m["xT"] = xT_c
            m["aT"] = aT_c
            in_maps.append(m)
            scatter.append(dst_pad)

        res = run_bass_kernel_spmd(nc, in_maps, list(range(N_CORES)))
        for ci in range(N_CORES):
            vals = np.asarray(res.results[ci]["out"]).reshape(-1)
            dst = scatter[ci]
            valid = dst >= 0
            out_full[dst[valid]] = vals[valid]

    return out_full.reshape(nb, na)

